# revision 1
# baseline (speedup 1.0000x reference)
"""Trainium2 Bass kernel for nn_Capsule: capsule layer with 3 dynamic-routing
iterations.

    u_hat = einsum('bip,iodp->biod', x, W)   # [64, 2048, 32, 32]
    3x routing: c = softmax(b, axis=2); s = sum_i c*u_hat; v = squash(s);
                b += sum_d v*u_hat

Strategy: shard in_caps (i) across 8 cores (256 each). W-shard and a
block-diagonalized x are SBUF-resident; u_hat is (re)computed on the tensor
engine each routing iteration, two capsules at a time, as
[K=32 (2i x 16p), M=128 (2i x 64b)] x [K=32, N=512 od] matmuls with
block-diagonal x as the stationary operand, spread over 4 PE row-groups.
Iteration 0 (uniform c) accumulates u directly in PSUM. Iterations 1-2
consume u tiles from PSUM on the vector engine: agreement = reduce_d(u*v),
logits update + softmax (ACT exp), s += c*u. Per-core s partials are
AllReduced after iters 0 and 1; the final iteration's partials are summed
and squashed on the host.
"""

import numpy as np

B, IN_CAPS, IN_DIM = 64, 2048, 16
NUM_CAPS, DIM_CAPS = 32, 32
OD = NUM_CAPS * DIM_CAPS  # 1024
ROUTING_ITERS = 3
EPS = 1e-7

N_CORES = 8
I_LOC = IN_CAPS // N_CORES       # 256
N_PAIRS = I_LOC // 2             # 128
N_RG = 4                         # PE row groups
N_PJ = N_PAIRS // N_RG           # 32 pairs per row group

_CACHE = {}


def _build_nc(stage=3):
    # stage 0: iter0 partial s0 only; 1: +AllReduce+squash (out=vrep);
    # 2: +iter1 (out=acc, no 2nd AllReduce); 3: full kernel.
    import concourse.bacc as bacc
    import concourse.bass as bass
    import concourse.tile as tile
    from concourse import mybir

    f32 = mybir.dt.float32
    Alu = mybir.AluOpType
    Act = mybir.ActivationFunctionType
    AxX = mybir.AxisListType.X

    nc = bacc.Bacc("TRN2", target_bir_lowering=False, debug=False,
                   num_devices=N_CORES)

    bf16 = mybir.dt.bfloat16
    xbd_d = nc.dram_tensor("xbd", [128, N_PJ * 128], bf16,
                           kind="ExternalInput")
    wp_d = nc.dram_tensor("wp", [128, N_PJ * OD], bf16, kind="ExternalInput")
    out_d = nc.dram_tensor("out_sp2", [128, OD], f32, kind="ExternalOutput")

    def bcast_d(ap_2d, o_cnt):
        """View a [P, o_cnt] AP as [P, o_cnt, DIM_CAPS] with the last dim
        broadcast (step 0)."""
        return bass.AP(tensor=ap_2d.tensor, offset=ap_2d.offset,
                       ap=[list(ap_2d.ap[0]), list(ap_2d.ap[1]),
                           [0, DIM_CAPS]])

    def lhsT_of(xbd, rg, pj):
        return xbd[32 * rg:32 * rg + 32, 128 * pj:128 * pj + 128]

    def rhs_of(wp, rg, pj, h):
        return wp[32 * rg:32 * rg + 32,
                  OD * pj + 512 * h:OD * pj + 512 * h + 512]

    with tile.TileContext(nc) as tc:
        with (
            nc.allow_low_precision(reason="bf16 routing intermediates"),
            tc.tile_pool(name="big", bufs=1) as big,
            tc.tile_pool(name="work", bufs=3) as work,
            tc.tile_pool(name="small", bufs=1) as small,
            tc.tile_pool(name="dram", bufs=1, space="DRAM") as dram,
        ):
            xbd = big.tile([128, N_PJ * 128], bf16)
            wp = big.tile([128, N_PJ * OD], bf16)
            for rg in range(N_RG):
                sl = slice(32 * rg, 32 * rg + 32)
                nc.sync.dma_start(xbd[sl, :], xbd_d[sl, :])
                nc.sync.dma_start(wp[sl, :], wp_d[sl, :])

            bl = big.tile([128, N_PAIRS * NUM_CAPS], bf16)  # routing logits
            vrep = big.tile([128, OD], bf16)                # v replicated 2x
            ident = big.tile([128, 128], bf16)              # PE-accumulate id
            from concourse.masks import make_identity
            make_identity(nc, ident[:])
            eps_t = big.tile([64, 1], f32)
            nc.vector.memset(eps_t[:], EPS)

            ar_count = [0]

            # ---------------- iteration 0: s0 = (1/32) * sum_i u ----------
            with tc.tile_pool(name="ps0", bufs=1, space="PSUM") as ps0:
                # s0 sums over ALL i, so the full 128-row (4 rg x 2 ipar x
                # 16 p) block-diagonal lhsT works in one matmul per (pj, h):
                # the rg split is only needed when per-pair u must stay
                # separate.
                acc0 = [ps0.tile([128, 512], f32, name=f"acc0_{h}",
                                 tag=f"acc0_{h}") for h in range(2)]
                for pj in range(N_PJ):
                    for h in range(2):
                        nc.tensor.matmul(
                            acc0[h][:],
                            xbd[:, 128 * pj:128 * pj + 128],
                            wp[:, OD * pj + 512 * h:OD * pj + 512 * h + 512],
                            start=(pj == 0), stop=(pj == N_PJ - 1),
                        )
                # fold the two capsule slots
                s0 = small.tile([64, OD], f32, tag="sfold")
                tmpu = small.tile([64, OD], f32, tag="tmpu")
                for h in range(2):
                    th = work.tile([128, 512], f32, tag="thfold")
                    nc.scalar.copy(out=th[:], in_=acc0[h][:])
                    # fold the two capsule slots (partitions 0-63 + 64-127);
                    # DVE can't read two different base partitions, so shift
                    # the upper half down via SBUF->SBUF DMA first.
                    nc.sync.dma_start(tmpu[:, 512 * h:512 * h + 512],
                                      th[64:128, :])
                    nc.vector.tensor_add(out=s0[:, 512 * h:512 * h + 512],
                                         in0=th[0:64, :],
                                         in1=tmpu[:, 512 * h:512 * h + 512])
            # scale by 1/NUM_CAPS (uniform softmax weight)
            nc.scalar.mul(out=s0[:], in_=s0[:], mul=1.0 / NUM_CAPS)
            if stage == 0:
                nc.sync.dma_start(out_d[0:64, :], s0[:])

            def all_reduce(sp):
                if stage == 4:  # timing variant: skip collectives
                    return sp
                k = ar_count[0]
                ar_count[0] += 1
                ar_in = dram.tile([64, OD], f32, name=f"ar_in{k}",
                                  tag=f"ar_in{k}")
                ar_out = dram.tile([64, OD], f32, name=f"ar_out{k}",
                                   tag=f"ar_out{k}")
                nc.sync.dma_start(ar_in[:], sp[:])
                nc.gpsimd.collective_compute(
                    "AllReduce", Alu.add,
                    replica_groups=[list(range(N_CORES))],
                    ins=[ar_in.opt()], outs=[ar_out.opt()])
                sq = small.tile([64, OD], f32, tag="sfold")
                nc.sync.dma_start(sq[:], ar_out[:])
                return sq

            def squash_to_vrep(sq):
                """v = (n/(1+n)) * s / sqrt(n+eps), n = sum_d s^2; then
                replicate v into both partition halves of vrep."""
                ssq = small.tile([64, OD], f32, tag="tmpu")
                nc.vector.tensor_mul(out=ssq[:], in0=sq[:], in1=sq[:])
                n_t = small.tile([64, NUM_CAPS], f32, tag="n_t")
                # d-major layout: reduce over d (strided, AP dims [o, d])
                nc.vector.tensor_reduce(
                    out=n_t[:],
                    in_=bass.AP(tensor=ssq.tensor, offset=ssq[:].offset,
                                ap=[list(ssq[:].ap[0]), [1, NUM_CAPS],
                                    [NUM_CAPS, DIM_CAPS]]),
                    axis=AxX, op=Alu.add)
                sr = small.tile([64, NUM_CAPS], f32, tag="sr")
                # sqrt via exp(0.5*ln): Ln/Exp share an ACT table set, so no
                # mid-kernel table reloads (Sqrt lives in a different set)
                nc.scalar.activation(out=sr[:], in_=n_t[:], func=Act.Ln,
                                     bias=eps_t[:], scale=1.0)
                nc.scalar.activation(out=sr[:], in_=sr[:], func=Act.Exp,
                                     bias=0.0, scale=0.5)
                nc.vector.reciprocal(out=sr[:], in_=sr[:])   # 1/sqrt(n+eps)
                np1 = small.tile([64, NUM_CAPS], f32, tag="np1")
                nc.vector.tensor_scalar_add(out=np1[:], in0=n_t[:],
                                            scalar1=1.0)
                nc.vector.reciprocal(out=np1[:], in_=np1[:])  # 1/(1+n)
                fac = small.tile([64, NUM_CAPS], f32, tag="fac")
                nc.vector.tensor_mul(out=fac[:], in0=n_t[:], in1=np1[:])
                nc.vector.tensor_mul(out=fac[:], in0=fac[:], in1=sr[:])
                # v = s * fac (broadcast fac over the outer d dim)
                nc.vector.tensor_tensor(
                    out=vrep[0:64, :].rearrange("p (d o) -> p d o",
                                                d=DIM_CAPS),
                    in0=sq[:].rearrange("p (d o) -> p d o", d=DIM_CAPS),
                    in1=bass.AP(tensor=fac.tensor, offset=fac[:].offset,
                                ap=[list(fac[:].ap[0]), [0, DIM_CAPS],
                                    [1, NUM_CAPS]]),
                    op=Alu.mult)
                nc.sync.dma_start(vrep[64:128, :], vrep[0:64, :])

            if stage >= 1:
                sq = all_reduce(s0)
                squash_to_vrep(sq)
            if stage == 1:
                vr32 = work.tile([128, OD], f32, tag="vr32")
                nc.scalar.copy(out=vr32[:], in_=vrep[:])
                nc.sync.dma_start(out_d[:], vr32[:])

            # ---------------- iterations 1..2 -----------------------------
            last_it = ROUTING_ITERS if stage >= 3 else stage
            if stage == 4:
                last_it = ROUTING_ITERS
            with tc.tile_pool(name="ps", bufs=2, space="PSUM") as ps:
                N_QUADS = N_PAIRS // 2
                for it in range(1, last_it):
                    # s-partials accumulate in PSUM via identity-matmuls
                    accps = ps.tile([128, OD], f32, name="accps",
                                    tag="accps", bufs=1)
                    acc_started = [False, False]
                    cm_pend = []    # cm tiles awaiting PE accumulation
                    cm_ready = []   # previous batch, safe to emit on PE
                    pend = []

                    def emit_cmacc(last=False):
                        for j, cmr in enumerate(cm_ready):
                            for ch in range(4):
                                half = ch % 2
                                st = not acc_started[half]
                                acc_started[half] = True
                                nc.tensor.matmul(
                                    accps[:, 512 * half:512 * half + 512],
                                    ident[:],
                                    cmr[:, 512 * ch:512 * ch + 512],
                                    start=st,
                                    stop=(last and j == len(cm_ready) - 1
                                          and ch >= 2),
                                    skip_group_check=True,
                                )
                        cm_ready.clear()

                    for q in range(N_QUADS):
                        # a quad = 2 consecutive pairs (4 capsules); split
                        # the c*u multiplies DVE/GPSIMD — gpsimd ops carry a
                        # large fixed in-sim cost, so don't overload it
                        side = 0 if q % 8 < 7 else 1
                        eng = nc.vector if side == 0 else nc.gpsimd
                        ub = work.tile([128, 2 * OD], bf16, tag="ub", bufs=8)
                        for sub in range(2):
                            pair = 2 * q + sub
                            rg, pj = pair % N_RG, pair // N_RG
                            ups = ps.tile([128, OD], f32, name="ups",
                                          tag="ups")
                            for h in range(2):
                                nc.tensor.matmul(
                                    ups[:, 512 * h:512 * h + 512],
                                    lhsT_of(xbd, rg, pj),
                                    rhs_of(wp, rg, pj, h),
                                    start=True, stop=True,
                                    tile_position=(32 * rg, 0),
                                )
                            # evacuate u to SBUF as bf16 on ACT (the engine
                            # with the most timeline headroom) so DVE tensor
                            # ops run in 2x mode
                            nc.scalar.copy(out=ub[:, OD * sub:OD * (sub + 1)],
                                           in_=ups[:])
                        # agreement = sum_d u * v  (both pairs at once);
                        # free layout of u is (sub, d, o) -- d-major
                        m = work.tile([128, 2 * OD], bf16, tag="m", bufs=3)
                        nc.vector.tensor_tensor(
                            out=m[:].rearrange("p (s od) -> p s od", s=2),
                            in0=ub[:].rearrange("p (s od) -> p s od", s=2),
                            in1=bass.AP(tensor=vrep.tensor,
                                        offset=vrep[:].offset,
                                        ap=[list(vrep[:].ap[0]), [0, 2],
                                            [1, OD]]),
                            op=Alu.mult)
                        # fold the 4 d-blocks of each pair into PSUM on
                        # the PE (identity-matmul accumulate): [2048] -> [512]
                        mhps = ps.tile([128, 512], f32, name="mhps",
                                       tag="mhps", bufs=2)
                        for s_ in range(2):
                            for blk in range(4):
                                nc.tensor.matmul(
                                    mhps[:, 256 * s_:256 * s_ + 256],
                                    ident[:],
                                    m[:, 1024 * s_ + 256 * blk:
                                      1024 * s_ + 256 * blk + 256],
                                    start=(blk == 0), stop=(blk == 3),
                                    skip_group_check=True,
                                )
                        # reduce remaining 8 d-blocks: AP dims [s, o, d]
                        red_in = bass.AP(
                            tensor=mhps.tensor, offset=mhps[:].offset,
                            ap=[list(mhps[:].ap[0]), [256, 2], [1, NUM_CAPS],
                                [NUM_CAPS, 8]])
                        bsl = bl[:, NUM_CAPS * 2 * q:NUM_CAPS * 2 * (q + 1)]
                        if it == 1:
                            # b was zero: logits = agreement, written directly
                            nc.vector.tensor_reduce(
                                out=bsl, in_=red_in, axis=AxX, op=Alu.add)
                        else:
                            agr = work.tile([128, 2 * NUM_CAPS], bf16,
                                            tag="agr", bufs=6)
                            nc.vector.tensor_reduce(
                                out=agr[:], in_=red_in, axis=AxX, op=Alu.add)
                            nc.vector.tensor_add(out=bsl, in0=bsl,
                                                 in1=agr[:])
                        pend.append((q, ub, eng))
                        if q % 4 != 3:
                            continue
                        # softmax for the last two quads (4 pairs) at once;
                        # logits are small, so exp w/o max-subtraction is safe
                        b8 = bl[:, NUM_CAPS * 2 * (q - 3):
                                NUM_CAPS * 2 * (q + 1)]
                        ce = work.tile([128, 8 * NUM_CAPS], bf16, tag="ce",
                                       bufs=4)
                        zs = work.tile([128, 8], f32, tag="zs", bufs=4)
                        # one exp call for all 8 pairs (each separate call
                        # pays ~185ns of ACT access bubble + accum read);
                        # the softmax denominators come from a strided DVE
                        # reduce instead of ACT accum_out
                        nc.scalar.activation(out=ce[:], in_=b8, func=Act.Exp,
                                             bias=0.0, scale=1.0)
                        nc.vector.tensor_reduce(
                            out=zs[:],
                            in_=ce[:].rearrange("p (s o) -> p s o", s=8),
                            axis=AxX, op=Alu.add)
                        nc.vector.reciprocal(out=zs[:], in_=zs[:])
                        # c = e / Z  (broadcast 1/Z over o)
                        nc.vector.tensor_tensor(
                            out=ce[:].rearrange("p (s o) -> p s o", s=8),
                            in0=ce[:].rearrange("p (s o) -> p s o", s=8),
                            in1=bass.AP(tensor=zs.tensor, offset=zs[:].offset,
                                        ap=[list(zs[:].ap[0]), [1, 8],
                                            [0, NUM_CAPS]]),
                            op=Alu.mult)
                        # let the PE fold the previous batch's products
                        # into PSUM now (one batch late, so the in-order PE
                        # never stalls the u-matmul stream)
                        emit_cmacc()
                        # cm = c * u for both pending quads (c broadcast over
                        # the outer d dim -> innermost step 1 keeps 2x mode)
                        for j, (qq, uq, engq) in enumerate(pend):
                            csl = bass.AP(
                                tensor=ce.tensor,
                                offset=ce[:, 2 * NUM_CAPS * j:].offset,
                                ap=[list(ce[:].ap[0]), [NUM_CAPS, 2],
                                    [0, DIM_CAPS], [1, NUM_CAPS]])
                            cm = work.tile([128, 2 * OD], bf16, name="cm",
                                           tag="cm", bufs=10)
                            engq.tensor_tensor(
                                out=cm[:].rearrange("p (s d o) -> p s d o",
                                                    s=2, d=DIM_CAPS),
                                in0=uq[:].rearrange("p (s d o) -> p s d o",
                                                    s=2, d=DIM_CAPS),
                                in1=csl, op=Alu.mult)
                            cm_pend.append(cm)
                        pend = []
                        cm_ready.extend(cm_pend)
                        cm_pend = []
                    emit_cmacc(last=True)
                    # evacuate the PSUM s-partial and fold capsule slots
                    acc = work.tile([128, OD], f32, tag="accev", bufs=1)
                    nc.scalar.copy(out=acc[:], in_=accps[:])
                    if it < last_it - 1:
                        sp = small.tile([64, OD], f32, tag="sfold")
                        tmpu2 = small.tile([64, OD], f32, tag="tmpu")
                        nc.sync.dma_start(tmpu2[:], acc[64:128, :])
                        nc.vector.tensor_add(out=sp[:], in0=acc[0:64, :],
                                             in1=tmpu2[:])
                        sq = all_reduce(sp)
                        squash_to_vrep(sq)
                    else:
                        nc.sync.dma_start(out_d[:], acc[:])
    nc.compile()
    return nc


def _prep_inputs(x, W):
    """Build per-core xbd [128, N_PJ*128] and wp [128, N_PJ*OD] arrays."""
    import ml_dtypes
    bf16 = ml_dtypes.bfloat16
    ins = []
    for c in range(N_CORES):
        xc = x[:, c * I_LOC:(c + 1) * I_LOC, :]          # [64, 256, 16]
        Wc = W[c * I_LOC:(c + 1) * I_LOC]                # [256, 32, 32, 16]
        # i_loc = 8*pj + 2*rg + ipar
        xr = np.ascontiguousarray(
            xc.reshape(B, N_PJ, N_RG, 2, IN_DIM)
              .transpose(3, 2, 4, 1, 0))                 # [ipar,rg,p,pj,b]
        xbd = np.zeros((N_RG, 2, IN_DIM, N_PJ, 2, B), dtype=np.float32)
        xbd[:, 0, :, :, 0, :] = xr[0]
        xbd[:, 1, :, :, 1, :] = xr[1]
        xbd = xbd.reshape(128, N_PJ * 128).astype(bf16)
        wr = np.ascontiguousarray(
            Wc.reshape(N_PJ, N_RG, 2, NUM_CAPS, DIM_CAPS, IN_DIM)
              .transpose(1, 2, 5, 0, 4, 3)               # [rg,ipar,p,pj,d,o]
              .reshape(128, N_PJ * OD)).astype(bf16)
        ins.append({"xbd": xbd, "wp": wr})
    return ins


def _squash_np(s):
    n = np.sum(np.square(s), axis=-1, keepdims=True)
    return (n / (1.0 + n)) * (s / np.sqrt(n + EPS))


def kernel(x, W, _trace=False):
    from concourse.bass_utils import run_bass_kernel_spmd

    x = np.asarray(x, dtype=np.float32)
    W = np.asarray(W, dtype=np.float32)
    if "nc" not in _CACHE:
        _CACHE["nc"] = _build_nc()
    nc = _CACHE["nc"]
    in_maps = _prep_inputs(x, W)
    res = run_bass_kernel_spmd(nc, in_maps, core_ids=list(range(N_CORES)),
                               trace=_trace)
    _CACHE["last_result"] = res
    sp = np.stack([r["out_sp2"] for r in res.results])   # [8, 128, OD]
    s2 = sp[:, 0:64, :].sum(axis=0) + sp[:, 64:128, :].sum(axis=0)
    s2_od = s2.reshape(B, DIM_CAPS, NUM_CAPS).transpose(0, 2, 1)
    v = _squash_np(np.ascontiguousarray(s2_od))
    return v.astype(np.float32)

